# revision 47
# baseline (speedup 1.0000x reference)
"""Trainium2 Bass kernel: KV-memory retrieval (pool -> cosine kNN -> softmax gather).

Strategy (8 cores): shard the 65536-slot memory across cores (8192 keys/values
each) and the 256-image batch across cores (32 each) for pooling + output.

Pipeline (per core, single SPMD launch) — restructured from the phase-serial
baseline to overlap the collectives and key/value streams with compute:

  1. stream x (2 batches per DMA, sync queue) -> pool -> qTl [512, 32];
     local sum-of-squares row appended; AllGather [513, 32] -> all queries
     (the AG runs while keys stream + get transposed).
  2. keys stream behind x on the sync FIFO; per 512-block: DVE square-reduce
     -> ACT sqrt -> DVE recip -> DVE row-scale, PE transposes (is_transpose,
     exact fp32) into a 12-block kT ring.  Transposes for the first 12 blocks
     sit ahead of all matmul1 in the PE queue, so they run under the AG1
     collective.
  3. matmul1 fp32 (exact — selection changes are catastrophic: one swapped
     top-32 index costs ~1.5e-2 rel err) qT.T @ kT -> sim f32 [256, 8192],
     per-block top-16 candidates (max8 rounds).
  4. local top-32 -> AllGather candidates (gpsimd queue) -> global top-32,
     threshold t, softmax stats (gmax, Z folded into exp bias).
  5. dense w = exp(sim*rinv + bias) * (sim >= t)  (all f32, exact STT mask).
  6. matmul2 in fp16 (values/weights rounded to fp16: ~5e-4 output rel err,
     no selection impact): PE transposes of w -> wT16; vals streamed f32 on
     the sync FIFO behind keys, cast to fp16 on DVE; accumulate
     vals.T @ wT -> matched.T [512, 256] in PSUM f32.
  7. transpose -> [256, 512], ReduceScatter(add) -> own batch shard [32, 512]
  8. broadcast over 784 spatial positions (DVE/ACT split), 2-batch out DMAs.

Queue routing (engine FIFOs are in-order; misplacement deadlocks or stalls):
  sync  : x, qag_in, keys 0-7, qag readback, keys 8-15, vals, mb, rs, out
  gpsimd: AG1, cd_in, AG-cand, gc readback, RS
"""

import math

import numpy as np

import concourse.bacc as bacc
import concourse.mybir as mybir
import concourse.tile as tile
from concourse.bass import ts
from concourse.bass_utils import run_bass_kernel_spmd
from concourse.masks import make_identity

F32 = mybir.dt.float32
F16 = mybir.dt.float16
AF = mybir.ActivationFunctionType
ALU = mybir.AluOpType

N_CORES = 8
NEG = -3.0e38

KT_BUFS = 11      # kT ring depth (blocks transposed ahead of matmul1)
KTB_BUFS = 2      # key-stream tiles in flight
VTB_BUFS = 3      # value-stream tiles in flight
VT16_BUFS = 8     # fp16 value tiles (eager-cast ring)


def build(B=256, C=512, HW=784, M=65536, K=32, n_cores=N_CORES, mb=512):
    """Build + bacc-compile the SPMD program. Returns nc."""
    BS = B // n_cores          # batches per core
    MS = M // n_cores          # memory slots per core
    CT = C // 128              # channel tiles (contraction tiles)
    BT = B // 128              # batch tiles
    BTW = 128
    assert B == 256 and C == 512 and K == 32 and M % (n_cores * mb) == 0
    NMB = MS // mb             # key blocks per core
    KTPB = mb // 128           # 128-row key tiles per block
    KPB = 16                   # candidates kept per 512-block (top-16)
    MT = MS // 128             # value tiles
    RG = [list(range(n_cores))]
    CC_AS = "Shared" if n_cores > 4 else "Local"
    XPD = 2                    # batches per x DMA
    OPD = 2                    # batches per out DMA

    nc = bacc.Bacc("TRN2", target_bir_lowering=False, debug=False,
                   num_devices=n_cores)

    xs = nc.dram_tensor("xs", [BS, C, HW], F32, kind="ExternalInput").ap()
    keys = nc.dram_tensor("keys", [MS, C], F32, kind="ExternalInput").ap()
    vals = nc.dram_tensor("vals", [MS, C], F32, kind="ExternalInput").ap()
    out = nc.dram_tensor("out", [BS, C, HW], F32, kind="ExternalOutput").ap()

    with tile.TileContext(nc) as tc:
        with (
            tc.tile_pool(name="consts", bufs=1) as consts,
            tc.tile_pool(name="persist", bufs=1) as persist,
            tc.tile_pool(name="dram", bufs=1, space="DRAM") as dram,
        ):
            identity = consts.tile([128, 128], F32)
            make_identity(nc, identity)
            ones_col = consts.tile([128, 1], F32)
            nc.vector.memset(ones_col, 1.0)
            ones_hw = consts.tile([128, HW], F32)
            nc.vector.memset(ones_hw, 1.0)

            sim = [persist.tile([BTW, MS], F32, name=f"sim{i}")
                   for i in range(BT)]
            cand = [persist.tile([BTW, NMB * KPB], F32, name=f"cand{i}")
                    for i in range(BT)]
            g32 = [persist.tile([BTW, K], F32, name=f"g32{i}")
                   for i in range(BT)]
            rinv = [persist.tile([BTW, 1], F32, name=f"rinv{i}")
                    for i in range(BT)]
            bias2 = [persist.tile([BTW, 1], F32, name=f"bias2{i}")
                     for i in range(BT)]
            nb_l = [persist.tile([BTW, 1], F32, name=f"nb_l{i}")
                    for i in range(BT)]
            rowfix = [persist.tile([BTW, 1], F32, name=f"rowfix{i}")
                      for i in range(BT)]
            qTt = persist.tile([128, CT, B], F32, name="qTt")
            qTl = persist.tile([128, CT, BS], F32, name="qTl")
            qn_row = persist.tile([1, B], F32, name="qn_row")
            ri_row = persist.tile([1, B], F32, name="ri_row")
            mT = persist.tile([128, CT, B], F32, name="mT")
            mTmy = [persist.tile([128, BS], F32, name=f"mTmy{i}")
                    for i in range(CT)]

            BS2 = BS // 2
            qag_in = [dram.tile([C, BS2], F32, name=f"qag_in{h}")
                      for h in range(2)]
            qag_out = [dram.tile([n_cores, C, BS2], F32, addr_space=CC_AS,
                                 name=f"qag_out{h}")
                       for h in range(2)]
            cd_in = dram.tile([B, K], F32)
            cd_out = dram.tile([n_cores, B, K], F32, addr_space=CC_AS)
            mb_dram = dram.tile([B, C], F32)
            rs_out = dram.tile([BS, C], F32)

            def emit_ag1(h):
                # AllGather queries for batch half h; dispatched early so the
                # ~40us collective dispatch latency hides under pooling.
                for ct in range(CT):
                    nc.sync.dma_start(
                        out=qag_in[h][ts(ct, 128), :],
                        in_=qTl[:, ct, h * BS2:(h + 1) * BS2])
                nc.gpsimd.collective_compute(
                    "AllGather", ALU.bypass, replica_groups=RG,
                    ins=[qag_in[h].opt()], outs=[qag_out[h].opt()])

            # ---------------- Phase P: pool x -> qTl + local ssq ----------
            hw_a = int(math.isqrt(HW))
            CTH = CT // 2
            with (
                tc.tile_pool(name="poolP", bufs=1) as pP,
            ):
                for xi in range(BS // XPD):
                    if xi * XPD == BS2:
                        emit_ag1(0)
                    xt = pP.tile([128, XPD, CT, HW], F32, tag="xt", bufs=2)
                    nc.sync.dma_start(
                        out=xt,
                        in_=xs[xi * XPD:(xi + 1) * XPD].rearrange(
                            "b (ct p) hw -> p b ct hw", p=128))
                    for bs_ in range(XPD):
                        b = xi * XPD + bs_
                        # DVE: first half of channel tiles, two-stage reduce
                        xp = pP.tile([128, CTH, HW // hw_a], F32, tag="xp",
                                     bufs=2)
                        nc.vector.tensor_reduce(
                            out=xp,
                            in_=xt[:, bs_, 0:CTH].rearrange(
                                "p ct (a b) -> p ct a b", a=HW // hw_a),
                            axis=mybir.AxisListType.X, op=ALU.add)
                        xq = pP.tile([128, CTH], F32, tag="xq", bufs=2)
                        nc.vector.tensor_reduce(
                            out=xq, in_=xp,
                            axis=mybir.AxisListType.X, op=ALU.add)
                        for ct in range(CTH):
                            nc.vector.tensor_copy(qTl[:, ct, b:b + 1],
                                                  xq[:, ct:ct + 1])
                        # ACT: second half via copy-accumulate
                        for ct in range(CTH, CT):
                            xsc = pP.tile([128, HW], F32, tag="xsc", bufs=2)
                            nc.scalar.activation(
                                xsc, xt[:, bs_, ct], AF.Copy,
                                accum_out=qTl[:, ct, b:b + 1])
            # ---------------- AG1b: second batch half ----------------
            emit_ag1(1)

            # ---------------- Phase K: keys -> kT ring; matmul1 + topk ----
            with (
                tc.tile_pool(name="poolK", bufs=1) as pK,
                tc.tile_pool(name="psumK", bufs=1, space="PSUM") as psK,
            ):
                pkt = [psK.tile([128, mb], F32, tag=f"pkt{dt}",
                                name=f"pkt{dt}") for dt in range(CT)]
                kT_tiles = {}
                copy_flip = [0]

                def emit_mm1(j):
                    kTt = kT_tiles.pop(j)
                    for bt in range(BT):
                        psim = psK.tile([BTW, mb], F32, tag="psim", bufs=3)
                        for dt in range(CT):
                            nc.tensor.matmul(
                                psim, lhsT=qTt[:, dt, ts(bt, BTW)],
                                rhs=kTt[:, dt],
                                start=(dt == 0), stop=(dt == CT - 1),
                                skip_group_check=True)
                        sblk = sim[bt][:, ts(j, mb)]
                        if copy_flip[0] % 2 == 0:
                            nc.vector.tensor_copy(sblk, psim)
                        else:
                            nc.scalar.copy(sblk, psim)
                        copy_flip[0] += 1
                        c8a = cand[bt][:, j * KPB:j * KPB + 8]
                        c8b = cand[bt][:, j * KPB + 8:j * KPB + 16]
                        nc.vector.max(c8a, sblk)
                        scr = pK.tile([BTW, mb], F32, tag="scr", bufs=1)
                        nc.vector.match_replace(
                            scr, in_to_replace=c8a, in_values=sblk,
                            imm_value=NEG)
                        nc.vector.max(c8b, scr)

                for mbi in range(NMB):
                    if mbi >= KT_BUFS:
                        emit_mm1(mbi - KT_BUFS)
                    ktb = pK.tile([128, KTPB, C], F32, tag="ktb",
                                  bufs=KTB_BUFS)
                    nc.sync.dma_start(
                        out=ktb,
                        in_=keys[mbi * mb:(mbi + 1) * mb].rearrange(
                            "(kt p) c -> p kt c", p=128))
                    kTt = pK.tile([128, CT, mb], F32, tag="kT",
                                  bufs=KT_BUFS)
                    kT_tiles[mbi] = kTt
                    for kt in range(KTPB):
                        ktile = ktb[:, kt]
                        kts = pK.tile([128, C], F32, tag="kts", bufs=2)
                        ssk = pK.tile([128, 1], F32, tag="ssk", bufs=2)
                        # kts doubles as the junk squares output here;
                        # the row-scale below overwrites it.
                        nc.scalar.activation(kts, ktile, AF.Square,
                                             accum_out=ssk)
                        kn = pK.tile([128, 1], F32, tag="kn", bufs=2)
                        nc.scalar.sqrt(kn, ssk)
                        rk = pK.tile([128, 1], F32, tag="rk", bufs=2)
                        nc.vector.reciprocal(rk, kn)
                        nc.vector.tensor_scalar_mul(kts, ktile, rk)
                        for dt in range(CT):
                            nc.tensor.matmul(
                                pkt[dt][:, ts(kt, 128)],
                                lhsT=kts[:, ts(dt, 128)], rhs=identity,
                                is_transpose=True,
                                start=True, stop=True, skip_group_check=True)
                    for dt in range(CT):
                        if dt % 2 == 0:
                            nc.vector.tensor_copy(kTt[:, dt], pkt[dt])
                        else:
                            nc.scalar.copy(kTt[:, dt], pkt[dt])
                    if mbi == NMB - 1:
                        # qag readback after all key DMAs so it never blocks
                        # the key stream (AG1b completes before keys drain).
                        for h in range(2):
                            for r in range(n_cores):
                                nc.sync.dma_start(
                                    out=qTt[:, :,
                                            r * BS + h * BS2:
                                            r * BS + (h + 1) * BS2],
                                    in_=qag_out[h][r].rearrange(
                                        "(ct p) b -> p ct b", p=128))
                for j in range(NMB - KT_BUFS, NMB):
                    emit_mm1(j)

            # ---------------- value stream (sync FIFO, behind keys) -------
            # Only the first VTB_BUFS value DMAs are issued ahead of the
            # candidate exchange: they fill fresh buffers and cannot stall
            # the FIFO.  The rest are emitted after the gc readback so their
            # buffer-reuse waits (on the W-phase fp16 casts) cannot block
            # cd_in/gc, which the W phase depends on.
            def emit_vtb(pV, vtbs, g):
                vtb = pV.tile([128, KTPB, C], F32, tag="vtb",
                              bufs=VTB_BUFS)
                nc.sync.dma_start(
                    out=vtb,
                    in_=vals[g * mb:(g + 1) * mb].rearrange(
                        "(kt p) c -> p kt c", p=128))
                vtbs.append(vtb)

            with tc.tile_pool(name="poolV", bufs=1) as pV:
                vtbs = []
                for g in range(VTB_BUFS):
                    emit_vtb(pV, vtbs, g)
                wexp = [pV.tile([BTW, MS], F32, name=f"wexp{i}")
                        for i in range(BT)]
                vt16s = []

                # ------------- Phase G: global top-K + softmax stats ------
                with (
                    tc.tile_pool(name="poolG", bufs=1) as pG,
                    tc.tile_pool(name="psumG", bufs=1, space="PSUM") as psG,
                ):
                    R = K // 8
                    # Q: rinv from the gathered queries
                    pss = psG.tile([1, B], F32, tag="pss")
                    for ct in range(CT):
                        qsq = pG.tile([128, B], F32, tag="qsq", bufs=2)
                        nc.scalar.square(qsq, qTt[:, ct])
                        nc.tensor.matmul(pss, lhsT=ones_col, rhs=qsq,
                                         start=(ct == 0), stop=(ct == CT - 1))
                    nc.scalar.sqrt(qn_row, pss)
                    nc.vector.reciprocal(ri_row, qn_row)
                    for bt in range(BT):
                        psum_rt = psG.tile([BTW, 1], F32, tag="rt", bufs=2)
                        nc.tensor.matmul(
                            psum_rt, lhsT=ri_row[0:1, ts(bt, BTW)],
                            rhs=ones_col[0:1, 0:1], start=True, stop=True)
                        nc.vector.tensor_copy(rinv[bt], psum_rt)
                        loc = pG.tile([BTW, K], F32, tag="loc", bufs=2)
                        scr2 = pG.tile([BTW, NMB * KPB], F32, tag="scr2",
                                       bufs=1)
                        cur = cand[bt]
                        for r in range(R):
                            nc.vector.max(loc[:, r * 8:(r + 1) * 8], cur)
                            if r < R - 1:
                                nc.vector.match_replace(
                                    scr2,
                                    in_to_replace=loc[:, r * 8:(r + 1) * 8],
                                    in_values=cur, imm_value=NEG)
                                cur = scr2
                        nc.sync.dma_start(out=cd_in[ts(bt, BTW), :],
                                          in_=loc)
                        # local softmax bias: nb_l = -lmax * rinv
                        nc.vector.tensor_mul(nb_l[bt], loc[:, 0:1], rinv[bt])
                        nc.vector.tensor_scalar_mul(nb_l[bt], nb_l[bt], -1.0)
                    # rest of the value stream: on the sync FIFO after cd_in,
                    # paced by the eager casts below — fills the AG-cand
                    # latency window with the value DMAs.
                    for g in range(VTB_BUFS, MT // KTPB):
                        emit_vtb(pV, vtbs, g)
                    # exp with LOCAL stats — runs under the AG-cand latency;
                    # the global correction folds into rowfix (phase O).
                    for bt in range(BT):
                        nc.scalar.activation(wexp[bt], sim[bt], AF.Exp,
                                             bias=nb_l[bt], scale=rinv[bt])
                    # eager fp16 casts: the DVE is idle here anyway until the
                    # gc data lands, so these are free — and they unblock the
                    # value stream into the AG-cand window.
                    for g in range(MT // KTPB):
                        vt16 = pV.tile([128, KTPB, C], F16, tag="vt16",
                                       bufs=VT16_BUFS, name=f"vt16_{g}")
                        nc.vector.tensor_copy(vt16, vtbs[g])
                        vt16s.append(vt16)
                    nc.gpsimd.collective_compute(
                        "AllGather", ALU.bypass, replica_groups=RG,
                        ins=[cd_in.opt()], outs=[cd_out.opt()])
                    for bt in range(BT):
                        gc = pG.tile([BTW, n_cores * K], F32, tag="gc",
                                     bufs=1)
                        nc.gpsimd.dma_start(
                            out=gc.rearrange("p (r k) -> p r k", r=n_cores),
                            in_=cd_out[:, ts(bt, BTW), :].rearrange(
                                "r p k -> p r k"))
                        scr3 = pG.tile([BTW, n_cores * K], F32, tag="scr3",
                                       bufs=2)
                        cur = gc
                        for r in range(R):
                            nc.vector.max(g32[bt][:, r * 8:(r + 1) * 8], cur)
                            if r < R - 1:
                                nc.vector.match_replace(
                                    scr3,
                                    in_to_replace=g32[bt][:,
                                                          r * 8:(r + 1) * 8],
                                    in_values=cur, imm_value=NEG)
                                cur = scr3
                        # stats: nbg = -gmax*rinv ; Z = sum exp((g-gmax)*rinv)
                        # rowfix = exp(nbg - nb_l - lnZ) applied in phase O
                        nbg = pG.tile([BTW, 1], F32, tag="nbg", bufs=2)
                        nc.vector.tensor_mul(nbg, g32[bt][:, 0:1], rinv[bt])
                        nc.vector.tensor_scalar_mul(nbg, nbg, -1.0)
                        ex = pG.tile([BTW, K], F32, tag="ex", bufs=2)
                        zz = pG.tile([BTW, 1], F32, tag="zz", bufs=2)
                        nc.scalar.activation(ex, g32[bt][:, 0:K], AF.Exp,
                                             bias=nbg, scale=rinv[bt],
                                             accum_out=zz)
                        lnz = pG.tile([BTW, 1], F32, tag="lnz", bufs=2)
                        nc.scalar.activation(lnz, zz, AF.Ln)
                        nc.vector.tensor_sub(bias2[bt], nbg, nb_l[bt])
                        nc.vector.tensor_sub(bias2[bt], bias2[bt], lnz)
                        nc.scalar.activation(rowfix[bt], bias2[bt], AF.Exp)

                # ------------- Phase W: dense weights + matmul2 (fp16) ----
                with (
                    tc.tile_pool(name="poolW", bufs=1) as pW,
                    tc.tile_pool(name="psumW", bufs=1, space="PSUM") as psW,
                ):
                    for bt in range(BT):
                        nc.vector.scalar_tensor_tensor(
                            out=wexp[bt], in0=sim[bt],
                            scalar=g32[bt][:, K - 1:K], in1=wexp[bt],
                            op0=ALU.is_ge, op1=ALU.mult)
                    pm = [psW.tile([128, B], F32, tag=f"pm{dt}",
                                   name=f"pm{dt}") for dt in range(CT)]
                    for mt in range(MT):
                        g, kt = mt // KTPB, mt % KTPB
                        vt16 = vt16s[g]
                        pwt = psW.tile([128, B], F32, tag="pwt", bufs=3)
                        for bt in range(BT):
                            nc.tensor.matmul(
                                pwt[:, ts(bt, BTW)],
                                lhsT=wexp[bt][:, ts(mt, 128)],
                                rhs=identity, is_transpose=True,
                                start=True, stop=True, skip_group_check=True)
                        wT16 = pW.tile([128, B], F16, tag="wT16", bufs=3)
                        if mt % 2 == 0:
                            nc.vector.tensor_copy(wT16, pwt)
                        else:
                            nc.scalar.copy(wT16, pwt)
                        for dt in range(CT):
                            nc.tensor.matmul(
                                pm[dt], lhsT=vt16[:, kt, ts(dt, 128)],
                                rhs=wT16,
                                start=(mt == 0), stop=(mt == MT - 1),
                                skip_group_check=True)
                    for dt in range(CT):
                        nc.any.tensor_copy(mT[:, dt], pm[dt])

            # ---------------- Phase O: reduce-scatter + broadcast out -----
            with (
                tc.tile_pool(name="poolO", bufs=1) as pO,
                tc.tile_pool(name="psumO", bufs=1, space="PSUM") as psO,
            ):
                for bt in range(BT):
                    pmb = psO.tile([BTW, C], F32, tag="pmb", bufs=2)
                    for dt in range(CT):
                        nc.tensor.matmul(
                            pmb[:, ts(dt, 128)],
                            lhsT=mT[:, dt, ts(bt, BTW)],
                            rhs=identity, is_transpose=True,
                            start=True, stop=True, skip_group_check=True)
                    mrow = pO.tile([BTW, C], F32, tag="mrow", bufs=2)
                    # fold the local->global softmax correction in here
                    nc.scalar.mul(mrow, pmb, rowfix[bt])
                    nc.sync.dma_start(out=mb_dram[ts(bt, BTW), :], in_=mrow)
                nc.gpsimd.collective_compute(
                    "ReduceScatter", ALU.add, replica_groups=RG,
                    ins=[mb_dram.opt()], outs=[rs_out.opt()])
                mmy = pO.tile([BS, C], F32, tag="mmy", bufs=1)
                nc.sync.dma_start(out=mmy, in_=rs_out)
                for dt in range(CT):
                    pmt = psO.tile([128, BS], F32, tag="pmt", bufs=2)
                    nc.tensor.matmul(
                        pmt, lhsT=mmy[:, ts(dt, 128)],
                        rhs=identity[0:BS, 0:BS], is_transpose=True,
                        start=True, stop=True, skip_group_check=True)
                    nc.any.tensor_copy(mTmy[dt], pmt)
                for oi in range(BS // OPD):
                    ot = pO.tile([128, OPD, CT, HW], F32, tag="ot", bufs=2)
                    for bs_ in range(OPD):
                        b = oi * OPD + bs_
                        for dt in range(CT):
                            col = mTmy[dt][:, b:b + 1]
                            if dt < CT // 2:
                                nc.vector.tensor_scalar_mul(
                                    ot[:, bs_, dt], ones_hw, col)
                            else:
                                nc.scalar.mul(ot[:, bs_, dt], ones_hw, col)
                    nc.sync.dma_start(
                        out=out[oi * OPD:(oi + 1) * OPD].rearrange(
                            "b (ct p) hw -> p b ct hw", p=128),
                        in_=ot)

    nc.compile()
    return nc


_CACHE = {}
TRACE = False
LAST_RESULT = None


def _get(shape_key):
    if shape_key not in _CACHE:
        _CACHE[shape_key] = build(*shape_key)
    return _CACHE[shape_key]


def kernel(x, keys, values, topk, **_ignored):
    K = int(np.asarray(topk))
    B, C, H, W = x.shape
    M, D = keys.shape
    HW = H * W
    nc = _get((B, C, HW, M, K, N_CORES))
    BS, MS = B // N_CORES, M // N_CORES
    x3 = np.ascontiguousarray(x.reshape(B, C, HW)).astype(np.float32, copy=False)
    keys = np.ascontiguousarray(keys).astype(np.float32, copy=False)
    values = np.ascontiguousarray(values).astype(np.float32, copy=False)
    in_maps = [{
        "xs": x3[c * BS:(c + 1) * BS],
        "keys": keys[c * MS:(c + 1) * MS],
        "vals": values[c * MS:(c + 1) * MS],
    } for c in range(N_CORES)]
    global LAST_RESULT
    res = run_bass_kernel_spmd(nc, in_maps, core_ids=list(range(N_CORES)),
                               trace=TRACE)
    LAST_RESULT = res
    outs = [res.results[c]["out"] for c in range(N_CORES)]
    return np.concatenate(outs, axis=0).reshape(B, C, H, W)


# revision 50
# speedup vs baseline: 1.0083x; 1.0083x over previous
"""Trainium2 Bass kernel: KV-memory retrieval (pool -> cosine kNN -> softmax gather).

Strategy (8 cores): shard the 65536-slot memory across cores (8192 keys/values
each) and the 256-image batch across cores (32 each) for pooling + output.

Pipeline (per core, single SPMD launch) — restructured from the phase-serial
baseline to overlap the collectives and key/value streams with compute:

  1. stream x (2 batches per DMA, sync queue) -> pool -> qTl [512, 32];
     local sum-of-squares row appended; AllGather [513, 32] -> all queries
     (the AG runs while keys stream + get transposed).
  2. keys stream behind x on the sync FIFO; per 512-block: DVE square-reduce
     -> ACT sqrt -> DVE recip -> DVE row-scale, PE transposes (is_transpose,
     exact fp32) into a 12-block kT ring.  Transposes for the first 12 blocks
     sit ahead of all matmul1 in the PE queue, so they run under the AG1
     collective.
  3. matmul1 fp32 (exact — selection changes are catastrophic: one swapped
     top-32 index costs ~1.5e-2 rel err) qT.T @ kT -> sim f32 [256, 8192],
     per-block top-16 candidates (max8 rounds).
  4. local top-32 -> AllGather candidates (gpsimd queue) -> global top-32,
     threshold t, softmax stats (gmax, Z folded into exp bias).
  5. dense w = exp(sim*rinv + bias) * (sim >= t)  (all f32, exact STT mask).
  6. matmul2 in fp16 (values/weights rounded to fp16: ~5e-4 output rel err,
     no selection impact): PE transposes of w -> wT16; vals streamed f32 on
     the sync FIFO behind keys, cast to fp16 on DVE; accumulate
     vals.T @ wT -> matched.T [512, 256] in PSUM f32.
  7. transpose -> [256, 512], ReduceScatter(add) -> own batch shard [32, 512]
  8. broadcast over 784 spatial positions (DVE/ACT split), 2-batch out DMAs.

Queue routing (engine FIFOs are in-order; misplacement deadlocks or stalls):
  sync  : x, qag_in, keys 0-7, qag readback, keys 8-15, vals, mb, rs, out
  gpsimd: AG1, cd_in, AG-cand, gc readback, RS
"""

import math

import numpy as np

import concourse.bacc as bacc
import concourse.mybir as mybir
import concourse.tile as tile
from concourse.bass import ts
from concourse.bass_utils import run_bass_kernel_spmd
from concourse.masks import make_identity

F32 = mybir.dt.float32
F16 = mybir.dt.float16
AF = mybir.ActivationFunctionType
ALU = mybir.AluOpType

N_CORES = 8
NEG = -3.0e38

KT_BUFS = 11      # kT ring depth (blocks transposed ahead of matmul1)
KTB_BUFS = 3      # key-stream tiles in flight
VTB_BUFS = 4      # value-stream tiles in flight


def build(B=256, C=512, HW=784, M=65536, K=32, n_cores=N_CORES, mb=512):
    """Build + bacc-compile the SPMD program. Returns nc."""
    BS = B // n_cores          # batches per core
    MS = M // n_cores          # memory slots per core
    CT = C // 128              # channel tiles (contraction tiles)
    BT = B // 128              # batch tiles
    BTW = 128
    assert B == 256 and C == 512 and K == 32 and M % (n_cores * mb) == 0
    NMB = MS // mb             # key blocks per core
    KTPB = mb // 128           # 128-row key tiles per block
    KPB = 16                   # candidates kept per 512-block (top-16)
    MT = MS // 128             # value tiles
    RG = [list(range(n_cores))]
    CC_AS = "Shared" if n_cores > 4 else "Local"
    XPD = 2                    # batches per x DMA
    OPD = 2                    # batches per out DMA

    nc = bacc.Bacc("TRN2", target_bir_lowering=False, debug=False,
                   num_devices=n_cores)

    xs = nc.dram_tensor("xs", [BS, C, HW], F32, kind="ExternalInput").ap()
    keys = nc.dram_tensor("keys", [MS, C], F32, kind="ExternalInput").ap()
    vals = nc.dram_tensor("vals", [MS, C], F32, kind="ExternalInput").ap()
    out = nc.dram_tensor("out", [BS, C, HW], F32, kind="ExternalOutput").ap()

    with tile.TileContext(nc) as tc:
        with (
            tc.tile_pool(name="consts", bufs=1) as consts,
            tc.tile_pool(name="persist", bufs=1) as persist,
            tc.tile_pool(name="dram", bufs=1, space="DRAM") as dram,
        ):
            identity = consts.tile([128, 128], F32)
            make_identity(nc, identity)
            ones_col = consts.tile([128, 1], F32)
            nc.vector.memset(ones_col, 1.0)
            ones_hw = consts.tile([128, HW], F32)
            nc.vector.memset(ones_hw, 1.0)

            sim = [persist.tile([BTW, MS], F32, name=f"sim{i}")
                   for i in range(BT)]
            cand = [persist.tile([BTW, NMB * KPB], F32, name=f"cand{i}")
                    for i in range(BT)]
            g32 = [persist.tile([BTW, K], F32, name=f"g32{i}")
                   for i in range(BT)]
            rinv = [persist.tile([BTW, 1], F32, name=f"rinv{i}")
                    for i in range(BT)]
            bias2 = [persist.tile([BTW, 1], F32, name=f"bias2{i}")
                     for i in range(BT)]
            nb_l = [persist.tile([BTW, 1], F32, name=f"nb_l{i}")
                    for i in range(BT)]
            rowfix = [persist.tile([BTW, 1], F32, name=f"rowfix{i}")
                      for i in range(BT)]
            qTt = persist.tile([128, CT, B], F32, name="qTt")
            qTl = persist.tile([128, CT, BS], F32, name="qTl")
            qn_row = persist.tile([1, B], F32, name="qn_row")
            ri_row = persist.tile([1, B], F32, name="ri_row")
            mT = persist.tile([128, CT, B], F32, name="mT")
            mTmy = [persist.tile([128, BS], F32, name=f"mTmy{i}")
                    for i in range(CT)]

            BS2 = BS // 2
            qag_in = [dram.tile([C, BS2], F32, name=f"qag_in{h}")
                      for h in range(2)]
            qag_out = [dram.tile([n_cores, C, BS2], F32, addr_space=CC_AS,
                                 name=f"qag_out{h}")
                       for h in range(2)]
            cd_in = dram.tile([B, K], F32)
            cd_out = dram.tile([n_cores, B, K], F32, addr_space=CC_AS)
            mb_dram = dram.tile([B, C], F32)
            rs_out = dram.tile([BS, C], F32)

            def emit_ag1(h):
                # AllGather queries for batch half h; dispatched early so the
                # ~40us collective dispatch latency hides under pooling.
                for ct in range(CT):
                    nc.sync.dma_start(
                        out=qag_in[h][ts(ct, 128), :],
                        in_=qTl[:, ct, h * BS2:(h + 1) * BS2])
                nc.gpsimd.collective_compute(
                    "AllGather", ALU.bypass, replica_groups=RG,
                    ins=[qag_in[h].opt()], outs=[qag_out[h].opt()])

            # ---------------- Phase P: pool x -> qTl + local ssq ----------
            hw_a = int(math.isqrt(HW))
            CTH = CT // 2
            with (
                tc.tile_pool(name="poolP", bufs=1) as pP,
            ):
                for xi in range(BS // XPD):
                    if xi * XPD == BS2:
                        emit_ag1(0)
                    xt = pP.tile([128, XPD, CT, HW], F32, tag="xt", bufs=2)
                    nc.sync.dma_start(
                        out=xt,
                        in_=xs[xi * XPD:(xi + 1) * XPD].rearrange(
                            "b (ct p) hw -> p b ct hw", p=128))
                    for bs_ in range(XPD):
                        b = xi * XPD + bs_
                        # DVE: first half of channel tiles, two-stage reduce
                        xp = pP.tile([128, CTH, HW // hw_a], F32, tag="xp",
                                     bufs=2)
                        nc.vector.tensor_reduce(
                            out=xp,
                            in_=xt[:, bs_, 0:CTH].rearrange(
                                "p ct (a b) -> p ct a b", a=HW // hw_a),
                            axis=mybir.AxisListType.X, op=ALU.add)
                        xq = pP.tile([128, CTH], F32, tag="xq", bufs=2)
                        nc.vector.tensor_reduce(
                            out=xq, in_=xp,
                            axis=mybir.AxisListType.X, op=ALU.add)
                        for ct in range(CTH):
                            nc.vector.tensor_copy(qTl[:, ct, b:b + 1],
                                                  xq[:, ct:ct + 1])
                        # ACT: second half via copy-accumulate
                        for ct in range(CTH, CT):
                            xsc = pP.tile([128, HW], F32, tag="xsc", bufs=2)
                            nc.scalar.activation(
                                xsc, xt[:, bs_, ct], AF.Copy,
                                accum_out=qTl[:, ct, b:b + 1])
            # ---------------- AG1b: second batch half ----------------
            emit_ag1(1)

            # ---------------- Phase K: keys -> kT ring; matmul1 + topk ----
            with (
                tc.tile_pool(name="poolK", bufs=1) as pK,
                tc.tile_pool(name="psumK", bufs=1, space="PSUM") as psK,
            ):
                pkt = [psK.tile([128, mb], F32, tag=f"pkt{dt}",
                                name=f"pkt{dt}") for dt in range(CT)]
                kT_tiles = {}
                copy_flip = [0]

                def emit_mm1(j):
                    kTt = kT_tiles.pop(j)
                    for bt in range(BT):
                        psim = psK.tile([BTW, mb], F32, tag="psim", bufs=4)
                        for dt in range(CT):
                            nc.tensor.matmul(
                                psim, lhsT=qTt[:, dt, ts(bt, BTW)],
                                rhs=kTt[:, dt],
                                start=(dt == 0), stop=(dt == CT - 1),
                                skip_group_check=True)
                        sblk = sim[bt][:, ts(j, mb)]
                        if copy_flip[0] % 2 == 0:
                            nc.vector.tensor_copy(sblk, psim)
                        else:
                            nc.scalar.copy(sblk, psim)
                        copy_flip[0] += 1
                        c8a = cand[bt][:, j * KPB:j * KPB + 8]
                        c8b = cand[bt][:, j * KPB + 8:j * KPB + 16]
                        nc.vector.max(c8a, sblk)
                        scr = pK.tile([BTW, mb], F32, tag="scr", bufs=1)
                        nc.vector.match_replace(
                            scr, in_to_replace=c8a, in_values=sblk,
                            imm_value=NEG)
                        nc.vector.max(c8b, scr)

                for mbi in range(NMB):
                    if mbi >= KT_BUFS:
                        emit_mm1(mbi - KT_BUFS)
                    ktb = pK.tile([128, KTPB, C], F32, tag="ktb",
                                  bufs=KTB_BUFS)
                    nc.sync.dma_start(
                        out=ktb,
                        in_=keys[mbi * mb:(mbi + 1) * mb].rearrange(
                            "(kt p) c -> p kt c", p=128))
                    kTt = pK.tile([128, CT, mb], F32, tag="kT",
                                  bufs=KT_BUFS)
                    kT_tiles[mbi] = kTt
                    for kt in range(KTPB):
                        ktile = ktb[:, kt]
                        kts = pK.tile([128, C], F32, tag="kts", bufs=2)
                        ssk = pK.tile([128, 1], F32, tag="ssk", bufs=2)
                        # kts doubles as the junk squares output here;
                        # the row-scale below overwrites it.
                        nc.scalar.activation(kts, ktile, AF.Square,
                                             accum_out=ssk)
                        kn = pK.tile([128, 1], F32, tag="kn", bufs=2)
                        nc.scalar.sqrt(kn, ssk)
                        rk = pK.tile([128, 1], F32, tag="rk", bufs=2)
                        nc.vector.reciprocal(rk, kn)
                        nc.vector.tensor_scalar_mul(kts, ktile, rk)
                        for dt in range(CT):
                            nc.tensor.matmul(
                                pkt[dt][:, ts(kt, 128)],
                                lhsT=kts[:, ts(dt, 128)], rhs=identity,
                                is_transpose=True,
                                start=True, stop=True, skip_group_check=True)
                    for dt in range(CT):
                        if dt % 2 == 0:
                            nc.vector.tensor_copy(kTt[:, dt], pkt[dt])
                        else:
                            nc.scalar.copy(kTt[:, dt], pkt[dt])
                    if mbi == NMB - 1:
                        # qag readback after all key DMAs so it never blocks
                        # the key stream (AG1b completes before keys drain).
                        for h in range(2):
                            for r in range(n_cores):
                                nc.sync.dma_start(
                                    out=qTt[:, :,
                                            r * BS + h * BS2:
                                            r * BS + (h + 1) * BS2],
                                    in_=qag_out[h][r].rearrange(
                                        "(ct p) b -> p ct b", p=128))
                for j in range(NMB - KT_BUFS, NMB):
                    emit_mm1(j)

            # ---------------- value stream (sync FIFO, behind keys) -------
            # Only the first VTB_BUFS value DMAs are issued ahead of the
            # candidate exchange: they fill fresh buffers and cannot stall
            # the FIFO.  The rest are emitted after the gc readback so their
            # buffer-reuse waits (on the W-phase fp16 casts) cannot block
            # cd_in/gc, which the W phase depends on.
            def emit_vtb(pV, vtbs, g):
                vtb = pV.tile([128, KTPB, C], F32, tag="vtb",
                              bufs=VTB_BUFS)
                nc.sync.dma_start(
                    out=vtb,
                    in_=vals[g * mb:(g + 1) * mb].rearrange(
                        "(kt p) c -> p kt c", p=128))
                vtbs.append(vtb)

            with tc.tile_pool(name="poolV", bufs=1) as pV:
                vtbs = []
                for g in range(VTB_BUFS):
                    emit_vtb(pV, vtbs, g)
                wexp = [pV.tile([BTW, MS], F32, name=f"wexp{i}")
                        for i in range(BT)]

                # ------------- Phase G: global top-K + softmax stats ------
                with (
                    tc.tile_pool(name="poolG", bufs=1) as pG,
                    tc.tile_pool(name="psumG", bufs=1, space="PSUM") as psG,
                ):
                    R = K // 8
                    # Q: rinv from the gathered queries
                    qsq = pG.tile([128, CT, B], F32, tag="qsq")
                    nc.scalar.square(qsq, qTt)
                    pss = psG.tile([1, B], F32, tag="pss")
                    for ct in range(CT):
                        nc.tensor.matmul(pss, lhsT=ones_col, rhs=qsq[:, ct],
                                         start=(ct == 0), stop=(ct == CT - 1))
                    nc.scalar.sqrt(qn_row, pss)
                    nc.vector.reciprocal(ri_row, qn_row)
                    for bt in range(BT):
                        psum_rt = psG.tile([BTW, 1], F32, tag="rt", bufs=2)
                        nc.tensor.matmul(
                            psum_rt, lhsT=ri_row[0:1, ts(bt, BTW)],
                            rhs=ones_col[0:1, 0:1], start=True, stop=True)
                        nc.vector.tensor_copy(rinv[bt], psum_rt)
                        loc = pG.tile([BTW, K], F32, tag="loc", bufs=2)
                        scr2 = pG.tile([BTW, NMB * KPB], F32, tag="scr2",
                                       bufs=2)
                        cur = cand[bt]
                        for r in range(R):
                            nc.vector.max(loc[:, r * 8:(r + 1) * 8], cur)
                            if r < R - 1:
                                nc.vector.match_replace(
                                    scr2,
                                    in_to_replace=loc[:, r * 8:(r + 1) * 8],
                                    in_values=cur, imm_value=NEG)
                                cur = scr2
                        nc.sync.dma_start(out=cd_in[ts(bt, BTW), :],
                                          in_=loc)
                        # local softmax bias: nb_l = -lmax * rinv
                        nc.vector.tensor_mul(nb_l[bt], loc[:, 0:1], rinv[bt])
                        nc.vector.tensor_scalar_mul(nb_l[bt], nb_l[bt], -1.0)
                    # exp with LOCAL stats — runs under the AG-cand latency;
                    # the global correction folds into rowfix (phase O).
                    for bt in range(BT):
                        nc.scalar.activation(wexp[bt], sim[bt], AF.Exp,
                                             bias=nb_l[bt], scale=rinv[bt])
                    nc.gpsimd.collective_compute(
                        "AllGather", ALU.bypass, replica_groups=RG,
                        ins=[cd_in.opt()], outs=[cd_out.opt()])
                    for bt in range(BT):
                        gc = pG.tile([BTW, n_cores * K], F32, tag="gc",
                                     bufs=2)
                        nc.sync.dma_start(
                            out=gc.rearrange("p (r k) -> p r k", r=n_cores),
                            in_=cd_out[:, ts(bt, BTW), :].rearrange(
                                "r p k -> p r k"))
                        scr3 = pG.tile([BTW, n_cores * K], F32, tag="scr3",
                                       bufs=2)
                        cur = gc
                        for r in range(R):
                            nc.vector.max(g32[bt][:, r * 8:(r + 1) * 8], cur)
                            if r < R - 1:
                                nc.vector.match_replace(
                                    scr3,
                                    in_to_replace=g32[bt][:,
                                                          r * 8:(r + 1) * 8],
                                    in_values=cur, imm_value=NEG)
                                cur = scr3
                        # stats: nbg = -gmax*rinv ; Z = sum exp((g-gmax)*rinv)
                        # rowfix = exp(nbg - nb_l - lnZ) applied in phase O
                        nbg = pG.tile([BTW, 1], F32, tag="nbg", bufs=2)
                        nc.vector.tensor_mul(nbg, g32[bt][:, 0:1], rinv[bt])
                        nc.vector.tensor_scalar_mul(nbg, nbg, -1.0)
                        ex = pG.tile([BTW, K], F32, tag="ex", bufs=2)
                        zz = pG.tile([BTW, 1], F32, tag="zz", bufs=2)
                        nc.scalar.activation(ex, g32[bt][:, 0:K], AF.Exp,
                                             bias=nbg, scale=rinv[bt],
                                             accum_out=zz)
                        lnz = pG.tile([BTW, 1], F32, tag="lnz", bufs=2)
                        nc.scalar.activation(lnz, zz, AF.Ln)
                        nc.vector.tensor_sub(bias2[bt], nbg, nb_l[bt])
                        nc.vector.tensor_sub(bias2[bt], bias2[bt], lnz)
                        nc.scalar.activation(rowfix[bt], bias2[bt], AF.Exp)

                # rest of the value stream (reuse-gated; see emit_vtb note)
                for g in range(VTB_BUFS, MT // KTPB):
                    emit_vtb(pV, vtbs, g)

                # ------------- Phase W: dense weights + matmul2 (fp16) ----
                with (
                    tc.tile_pool(name="poolW", bufs=1) as pW,
                    tc.tile_pool(name="psumW", bufs=1, space="PSUM") as psW,
                ):
                    for bt in range(BT):
                        nc.vector.scalar_tensor_tensor(
                            out=wexp[bt], in0=sim[bt],
                            scalar=g32[bt][:, K - 1:K], in1=wexp[bt],
                            op0=ALU.is_ge, op1=ALU.mult)
                    pm = [psW.tile([128, B], F32, tag=f"pm{dt}",
                                   name=f"pm{dt}") for dt in range(CT)]
                    vt16 = None
                    for mt in range(MT):
                        g, kt = mt // KTPB, mt % KTPB
                        if kt == 0:
                            vt16 = pW.tile([128, KTPB, C], F16, tag="vt16",
                                           bufs=2)
                            nc.vector.tensor_copy(vt16, vtbs[g])
                        pwt = psW.tile([128, B], F32, tag="pwt", bufs=3)
                        for bt in range(BT):
                            nc.tensor.matmul(
                                pwt[:, ts(bt, BTW)],
                                lhsT=wexp[bt][:, ts(mt, 128)],
                                rhs=identity, is_transpose=True,
                                start=True, stop=True, skip_group_check=True)
                        wT16 = pW.tile([128, B], F16, tag="wT16", bufs=3)
                        if mt % 2 == 0:
                            nc.vector.tensor_copy(wT16, pwt)
                        else:
                            nc.scalar.copy(wT16, pwt)
                        for dt in range(CT):
                            nc.tensor.matmul(
                                pm[dt], lhsT=vt16[:, kt, ts(dt, 128)],
                                rhs=wT16,
                                start=(mt == 0), stop=(mt == MT - 1),
                                skip_group_check=True)
                    for dt in range(CT):
                        nc.any.tensor_copy(mT[:, dt], pm[dt])

            # ---------------- Phase O: reduce-scatter + broadcast out -----
            with (
                tc.tile_pool(name="poolO", bufs=1) as pO,
                tc.tile_pool(name="psumO", bufs=1, space="PSUM") as psO,
            ):
                for bt in range(BT):
                    pmb = psO.tile([BTW, C], F32, tag="pmb", bufs=2)
                    for dt in range(CT):
                        nc.tensor.matmul(
                            pmb[:, ts(dt, 128)],
                            lhsT=mT[:, dt, ts(bt, BTW)],
                            rhs=identity, is_transpose=True,
                            start=True, stop=True, skip_group_check=True)
                    mrow = pO.tile([BTW, C], F32, tag="mrow", bufs=2)
                    # fold the local->global softmax correction in here
                    nc.scalar.mul(mrow, pmb, rowfix[bt])
                    nc.sync.dma_start(out=mb_dram[ts(bt, BTW), :], in_=mrow)
                nc.gpsimd.collective_compute(
                    "ReduceScatter", ALU.add, replica_groups=RG,
                    ins=[mb_dram.opt()], outs=[rs_out.opt()])
                mmy = pO.tile([BS, C], F32, tag="mmy", bufs=1)
                nc.sync.dma_start(out=mmy, in_=rs_out)
                for dt in range(CT):
                    pmt = psO.tile([128, BS], F32, tag="pmt", bufs=2)
                    nc.tensor.matmul(
                        pmt, lhsT=mmy[:, ts(dt, 128)],
                        rhs=identity[0:BS, 0:BS], is_transpose=True,
                        start=True, stop=True, skip_group_check=True)
                    nc.any.tensor_copy(mTmy[dt], pmt)
                for oi in range(BS // OPD):
                    ot = pO.tile([128, OPD, CT, HW], F32, tag="ot", bufs=2)
                    for bs_ in range(OPD):
                        b = oi * OPD + bs_
                        for dt in range(CT):
                            col = mTmy[dt][:, b:b + 1]
                            if dt < CT // 2:
                                nc.vector.tensor_scalar_mul(
                                    ot[:, bs_, dt], ones_hw, col)
                            else:
                                nc.scalar.mul(ot[:, bs_, dt], ones_hw, col)
                    nc.sync.dma_start(
                        out=out[oi * OPD:(oi + 1) * OPD].rearrange(
                            "b (ct p) hw -> p b ct hw", p=128),
                        in_=ot)

    nc.compile()
    return nc


_CACHE = {}
TRACE = False
LAST_RESULT = None


def _get(shape_key):
    if shape_key not in _CACHE:
        _CACHE[shape_key] = build(*shape_key)
    return _CACHE[shape_key]


def kernel(x, keys, values, topk, **_ignored):
    K = int(np.asarray(topk))
    B, C, H, W = x.shape
    M, D = keys.shape
    HW = H * W
    nc = _get((B, C, HW, M, K, N_CORES))
    BS, MS = B // N_CORES, M // N_CORES
    x3 = np.ascontiguousarray(x.reshape(B, C, HW)).astype(np.float32, copy=False)
    keys = np.ascontiguousarray(keys).astype(np.float32, copy=False)
    values = np.ascontiguousarray(values).astype(np.float32, copy=False)
    in_maps = [{
        "xs": x3[c * BS:(c + 1) * BS],
        "keys": keys[c * MS:(c + 1) * MS],
        "vals": values[c * MS:(c + 1) * MS],
    } for c in range(N_CORES)]
    global LAST_RESULT
    res = run_bass_kernel_spmd(nc, in_maps, core_ids=list(range(N_CORES)),
                               trace=TRACE)
    LAST_RESULT = res
    outs = [res.results[c]["out"] for c in range(N_CORES)]
    return np.concatenate(outs, axis=0).reshape(B, C, H, W)


# revision 57
# speedup vs baseline: 1.0760x; 1.0672x over previous
"""Trainium2 Bass kernel: KV-memory retrieval (pool -> cosine kNN -> softmax gather).

Strategy (8 cores): shard the 65536-slot memory across cores (8192 keys/values
each) and the 256-image batch across cores (32 each) for pooling + output.

Pipeline (per core, single SPMD launch) — restructured from the phase-serial
baseline to overlap the collectives and key/value streams with compute:

  1. stream x (2 batches per DMA, sync queue) -> pool -> qTl [512, 32];
     local sum-of-squares row appended; AllGather [513, 32] -> all queries
     (the AG runs while keys stream + get transposed).
  2. keys stream behind x on the sync FIFO; per 512-block: DVE square-reduce
     -> ACT sqrt -> DVE recip -> DVE row-scale, PE transposes (is_transpose,
     exact fp32) into a 12-block kT ring.  Transposes for the first 12 blocks
     sit ahead of all matmul1 in the PE queue, so they run under the AG1
     collective.
  3. matmul1 fp32 (exact — selection changes are catastrophic: one swapped
     top-32 index costs ~1.5e-2 rel err) qT.T @ kT -> sim f32 [256, 8192],
     per-block top-16 candidates (max8 rounds).
  4. local top-32 -> AllGather candidates (gpsimd queue) -> global top-32,
     threshold t, softmax stats (gmax, Z folded into exp bias).
  5. dense w = exp(sim*rinv + bias) * (sim >= t)  (all f32, exact STT mask).
  6. matmul2 in fp16 (values/weights rounded to fp16: ~5e-4 output rel err,
     no selection impact): PE transposes of w -> wT16; vals streamed f32 on
     the sync FIFO behind keys, cast to fp16 on DVE; accumulate
     vals.T @ wT -> matched.T [512, 256] in PSUM f32.
  7. transpose -> [256, 512], ReduceScatter(add) -> own batch shard [32, 512]
  8. broadcast over 784 spatial positions (DVE/ACT split), 2-batch out DMAs.

Queue routing (engine FIFOs are in-order; misplacement deadlocks or stalls):
  sync  : x, qag_in, keys 0-7, qag readback, keys 8-15, vals, mb, rs, out
  gpsimd: AG1, cd_in, AG-cand, gc readback, RS
"""

import math

import numpy as np

import concourse.bacc as bacc
import concourse.mybir as mybir
import concourse.tile as tile
from concourse.bass import ts
from concourse.bass_utils import run_bass_kernel_spmd
from concourse.masks import make_identity

F32 = mybir.dt.float32
F16 = mybir.dt.float16
BF16 = mybir.dt.bfloat16
AF = mybir.ActivationFunctionType
ALU = mybir.AluOpType

N_CORES = 8
NEG = -3.0e38

KT_BUFS = 11      # kT ring depth (blocks transposed ahead of matmul1)
KTB_BUFS = 2      # key-stream tiles in flight
VTB_BUFS = 4      # value-stream tiles in flight


def build(B=256, C=512, HW=784, M=65536, K=32, n_cores=N_CORES, mb=512):
    """Build + bacc-compile the SPMD program. Returns nc."""
    BS = B // n_cores          # batches per core
    MS = M // n_cores          # memory slots per core
    CT = C // 128              # channel tiles (contraction tiles)
    BT = B // 128              # batch tiles
    BTW = 128
    assert B == 256 and C == 512 and K == 32 and M % (n_cores * mb) == 0
    NMB = MS // mb             # key blocks per core
    KTPB = mb // 128           # 128-row key tiles per block
    KPB = 16                   # candidates kept per 512-block (top-16)
    MT = MS // 128             # value tiles
    RG = [list(range(n_cores))]
    CC_AS = "Shared" if n_cores > 4 else "Local"
    XPD = 2                    # batches per x DMA
    OPD = 2                    # batches per out DMA

    nc = bacc.Bacc("TRN2", target_bir_lowering=False, debug=False,
                   num_devices=n_cores)

    xs = nc.dram_tensor("xs", [BS, C, HW], F32, kind="ExternalInput").ap()
    keys = nc.dram_tensor("keys", [MS, C], F32, kind="ExternalInput").ap()
    vals = nc.dram_tensor("vals", [MS, C], F32, kind="ExternalInput").ap()
    out = nc.dram_tensor("out", [BS, C, HW], F32, kind="ExternalOutput").ap()

    with tile.TileContext(nc) as tc:
        with (
            tc.tile_pool(name="consts", bufs=1) as consts,
            tc.tile_pool(name="persist", bufs=1) as persist,
            tc.tile_pool(name="dram", bufs=1, space="DRAM") as dram,
        ):
            identity = consts.tile([128, 128], F32)
            make_identity(nc, identity)
            ones_col = consts.tile([128, 1], F32)
            nc.vector.memset(ones_col, 1.0)
            ones_hw = consts.tile([128, HW], F32)
            nc.vector.memset(ones_hw, 1.0)
            identity16 = consts.tile([128, 128], BF16)
            nc.vector.tensor_copy(identity16, identity)

            sim = [persist.tile([BTW, MS], F32, name=f"sim{i}")
                   for i in range(BT)]
            cand = [persist.tile([BTW, NMB * KPB], F32, name=f"cand{i}")
                    for i in range(BT)]
            g32 = [persist.tile([BTW, K], F32, name=f"g32{i}")
                   for i in range(BT)]
            rinv = [persist.tile([BTW, 1], F32, name=f"rinv{i}")
                    for i in range(BT)]
            bias2 = [persist.tile([BTW, 1], F32, name=f"bias2{i}")
                     for i in range(BT)]
            nb_l = [persist.tile([BTW, 1], F32, name=f"nb_l{i}")
                    for i in range(BT)]
            rowfix = [persist.tile([BTW, 1], F32, name=f"rowfix{i}")
                      for i in range(BT)]
            qTt = persist.tile([128, CT, B], F32, name="qTt")
            qTl = persist.tile([128, CT, BS], F32, name="qTl")
            qn_row = persist.tile([1, B], F32, name="qn_row")
            ri_row = persist.tile([1, B], F32, name="ri_row")
            mT = persist.tile([128, CT, B], F32, name="mT")
            mTmy = [persist.tile([128, BS], F32, name=f"mTmy{i}")
                    for i in range(CT)]

            BS2 = BS // 2
            qag_in = [dram.tile([C, BS2], F32, name=f"qag_in{h}")
                      for h in range(2)]
            qag_out = [dram.tile([n_cores, C, BS2], F32, addr_space=CC_AS,
                                 name=f"qag_out{h}")
                       for h in range(2)]
            cd_in = dram.tile([B, K], F32)
            cd_out = dram.tile([n_cores, B, K], F32, addr_space=CC_AS)
            # bf16 ReduceScatter payload: halves the tail-path collective;
            # ~2e-3 error contribution, 10x under the gate
            mb_dram = dram.tile([B, C], BF16)
            rs_out = dram.tile([BS, C], BF16)

            def emit_ag1(h):
                # AllGather queries for batch half h; dispatched early so the
                # ~40us collective dispatch latency hides under pooling.
                for ct in range(CT):
                    nc.sync.dma_start(
                        out=qag_in[h][ts(ct, 128), :],
                        in_=qTl[:, ct, h * BS2:(h + 1) * BS2])
                nc.gpsimd.collective_compute(
                    "AllGather", ALU.bypass, replica_groups=RG,
                    ins=[qag_in[h].opt()], outs=[qag_out[h].opt()])

            # ---------------- Phase P: pool x -> qTl + local ssq ----------
            hw_a = int(math.isqrt(HW))
            CTH = CT // 2
            with (
                tc.tile_pool(name="poolP", bufs=1) as pP,
            ):
                for xi in range(BS // XPD):
                    if xi * XPD == BS2:
                        emit_ag1(0)
                    xt = pP.tile([128, XPD, CT, HW], F32, tag="xt", bufs=2)
                    nc.sync.dma_start(
                        out=xt,
                        in_=xs[xi * XPD:(xi + 1) * XPD].rearrange(
                            "b (ct p) hw -> p b ct hw", p=128))
                    for bs_ in range(XPD):
                        b = xi * XPD + bs_
                        # DVE: first half of channel tiles, two-stage reduce
                        xp = pP.tile([128, CTH, HW // hw_a], F32, tag="xp",
                                     bufs=2)
                        nc.vector.tensor_reduce(
                            out=xp,
                            in_=xt[:, bs_, 0:CTH].rearrange(
                                "p ct (a b) -> p ct a b", a=HW // hw_a),
                            axis=mybir.AxisListType.X, op=ALU.add)
                        xq = pP.tile([128, CTH], F32, tag="xq", bufs=2)
                        nc.vector.tensor_reduce(
                            out=xq, in_=xp,
                            axis=mybir.AxisListType.X, op=ALU.add)
                        for ct in range(CTH):
                            nc.vector.tensor_copy(qTl[:, ct, b:b + 1],
                                                  xq[:, ct:ct + 1])
                        # ACT: second half via copy-accumulate
                        for ct in range(CTH, CT):
                            xsc = pP.tile([128, HW], F32, tag="xsc", bufs=2)
                            nc.scalar.activation(
                                xsc, xt[:, bs_, ct], AF.Copy,
                                accum_out=qTl[:, ct, b:b + 1])
            # ---------------- AG1b: second batch half ----------------
            emit_ag1(1)

            # ---------------- Phase K: keys -> kT ring; matmul1 + topk ----
            with (
                tc.tile_pool(name="poolK", bufs=1) as pK,
                tc.tile_pool(name="psumK", bufs=1, space="PSUM") as psK,
            ):
                pkt = [psK.tile([128, mb], F32, tag=f"pkt{dt}",
                                name=f"pkt{dt}") for dt in range(CT)]
                kT_tiles = {}
                copy_flip = [0]

                def emit_mm1(j):
                    kTt = kT_tiles.pop(j)
                    for bt in range(BT):
                        psim = psK.tile([BTW, mb], F32, tag="psim", bufs=3)
                        for dt in range(CT):
                            nc.tensor.matmul(
                                psim, lhsT=qTt[:, dt, ts(bt, BTW)],
                                rhs=kTt[:, dt],
                                start=(dt == 0), stop=(dt == CT - 1),
                                skip_group_check=True)
                        sblk = sim[bt][:, ts(j, mb)]
                        if copy_flip[0] % 2 == 0:
                            nc.vector.tensor_copy(sblk, psim)
                        else:
                            nc.scalar.copy(sblk, psim)
                        copy_flip[0] += 1
                        c8a = cand[bt][:, j * KPB:j * KPB + 8]
                        c8b = cand[bt][:, j * KPB + 8:j * KPB + 16]
                        nc.vector.max(c8a, sblk)
                        scr = pK.tile([BTW, mb], F32, tag="scr", bufs=1)
                        nc.vector.match_replace(
                            scr, in_to_replace=c8a, in_values=sblk,
                            imm_value=NEG)
                        nc.vector.max(c8b, scr)

                for mbi in range(NMB):
                    if mbi >= KT_BUFS:
                        emit_mm1(mbi - KT_BUFS)
                    ktb = pK.tile([128, KTPB, C], F32, tag="ktb",
                                  bufs=KTB_BUFS)
                    nc.sync.dma_start(
                        out=ktb,
                        in_=keys[mbi * mb:(mbi + 1) * mb].rearrange(
                            "(kt p) c -> p kt c", p=128))
                    kTt = pK.tile([128, CT, mb], F32, tag="kT",
                                  bufs=KT_BUFS)
                    kT_tiles[mbi] = kTt
                    for kt in range(KTPB):
                        ktile = ktb[:, kt]
                        kts = pK.tile([128, C], F32, tag="kts", bufs=2)
                        ssk = pK.tile([128, 1], F32, tag="ssk", bufs=2)
                        # kts doubles as the junk squares output here;
                        # the row-scale below overwrites it.
                        nc.scalar.activation(kts, ktile, AF.Square,
                                             accum_out=ssk)
                        kn = pK.tile([128, 1], F32, tag="kn", bufs=2)
                        nc.scalar.sqrt(kn, ssk)
                        rk = pK.tile([128, 1], F32, tag="rk", bufs=2)
                        nc.vector.reciprocal(rk, kn)
                        nc.vector.tensor_scalar_mul(kts, ktile, rk)
                        for dt in range(CT):
                            nc.tensor.matmul(
                                pkt[dt][:, ts(kt, 128)],
                                lhsT=kts[:, ts(dt, 128)], rhs=identity,
                                is_transpose=True,
                                start=True, stop=True, skip_group_check=True)
                    for dt in range(CT):
                        if dt % 2 == 0:
                            nc.vector.tensor_copy(kTt[:, dt], pkt[dt])
                        else:
                            nc.scalar.copy(kTt[:, dt], pkt[dt])
                    if mbi == NMB - 1:
                        # qag readback after all key DMAs so it never blocks
                        # the key stream (AG1b completes before keys drain).
                        for h in range(2):
                            for r in range(n_cores):
                                nc.sync.dma_start(
                                    out=qTt[:, :,
                                            r * BS + h * BS2:
                                            r * BS + (h + 1) * BS2],
                                    in_=qag_out[h][r].rearrange(
                                        "(ct p) b -> p ct b", p=128))
                for j in range(NMB - KT_BUFS, NMB):
                    emit_mm1(j)

            # ---------------- value stream (sync FIFO, behind keys) -------
            # Only the first VTB_BUFS value DMAs are issued ahead of the
            # candidate exchange: they fill fresh buffers and cannot stall
            # the FIFO.  The rest are emitted after the gc readback so their
            # buffer-reuse waits (on the W-phase fp16 casts) cannot block
            # cd_in/gc, which the W phase depends on.
            def emit_vtb(pV, vtbs, g):
                vtb = pV.tile([128, KTPB, C], F32, tag="vtb",
                              bufs=VTB_BUFS)
                nc.sync.dma_start(
                    out=vtb,
                    in_=vals[g * mb:(g + 1) * mb].rearrange(
                        "(kt p) c -> p kt c", p=128))
                vtbs.append(vtb)

            with tc.tile_pool(name="poolV", bufs=1) as pV:
                vtbs = []
                for g in range(VTB_BUFS):
                    emit_vtb(pV, vtbs, g)
                wexp = [pV.tile([BTW, MS], F32, name=f"wexp{i}")
                        for i in range(BT)]

                # ------------- Phase G: global top-K + softmax stats ------
                with (
                    tc.tile_pool(name="poolG", bufs=1) as pG,
                    tc.tile_pool(name="psumG", bufs=1, space="PSUM") as psG,
                ):
                    R = K // 8
                    # Q: rinv from the gathered queries
                    qsq = pG.tile([128, CT, B], F32, tag="qsq")
                    nc.scalar.square(qsq, qTt)
                    pss = psG.tile([1, B], F32, tag="pss")
                    for ct in range(CT):
                        nc.tensor.matmul(pss, lhsT=ones_col, rhs=qsq[:, ct],
                                         start=(ct == 0), stop=(ct == CT - 1))
                    nc.scalar.sqrt(qn_row, pss)
                    nc.vector.reciprocal(ri_row, qn_row)
                    for bt in range(BT):
                        psum_rt = psG.tile([BTW, 1], F32, tag="rt", bufs=2)
                        nc.tensor.matmul(
                            psum_rt, lhsT=ri_row[0:1, ts(bt, BTW)],
                            rhs=ones_col[0:1, 0:1], start=True, stop=True)
                        nc.vector.tensor_copy(rinv[bt], psum_rt)
                        loc = pG.tile([BTW, K], F32, tag="loc", bufs=2)
                        scr2 = pG.tile([BTW, NMB * KPB], F32, tag="scr2",
                                       bufs=2)
                        cur = cand[bt]
                        for r in range(R):
                            nc.vector.max(loc[:, r * 8:(r + 1) * 8], cur)
                            if r < R - 1:
                                nc.vector.match_replace(
                                    scr2,
                                    in_to_replace=loc[:, r * 8:(r + 1) * 8],
                                    in_values=cur, imm_value=NEG)
                                cur = scr2
                        nc.sync.dma_start(out=cd_in[ts(bt, BTW), :],
                                          in_=loc)
                        # local softmax bias: nb_l = -lmax * rinv
                        nc.vector.tensor_mul(nb_l[bt], loc[:, 0:1], rinv[bt])
                        nc.vector.tensor_scalar_mul(nb_l[bt], nb_l[bt], -1.0)
                    # exp with LOCAL stats — runs under the AG-cand latency;
                    # the global correction folds into rowfix (phase O).
                    for bt in range(BT):
                        nc.scalar.activation(wexp[bt], sim[bt], AF.Exp,
                                             bias=nb_l[bt], scale=rinv[bt])
                    nc.gpsimd.collective_compute(
                        "AllGather", ALU.bypass, replica_groups=RG,
                        ins=[cd_in.opt()], outs=[cd_out.opt()])
                    for bt in range(BT):
                        gc = pG.tile([BTW, n_cores * K], F32, tag="gc",
                                     bufs=2)
                        nc.sync.dma_start(
                            out=gc.rearrange("p (r k) -> p r k", r=n_cores),
                            in_=cd_out[:, ts(bt, BTW), :].rearrange(
                                "r p k -> p r k"))
                        scr3 = pG.tile([BTW, n_cores * K], F32, tag="scr3",
                                       bufs=2)
                        cur = gc
                        for r in range(R):
                            nc.vector.max(g32[bt][:, r * 8:(r + 1) * 8], cur)
                            if r < R - 1:
                                nc.vector.match_replace(
                                    scr3,
                                    in_to_replace=g32[bt][:,
                                                          r * 8:(r + 1) * 8],
                                    in_values=cur, imm_value=NEG)
                                cur = scr3
                        # stats: nbg = -gmax*rinv ; Z = sum exp((g-gmax)*rinv)
                        # rowfix = exp(nbg - nb_l - lnZ) applied in phase O
                        nbg = pG.tile([BTW, 1], F32, tag="nbg", bufs=2)
                        nc.vector.tensor_mul(nbg, g32[bt][:, 0:1], rinv[bt])
                        nc.vector.tensor_scalar_mul(nbg, nbg, -1.0)
                        ex = pG.tile([BTW, K], F32, tag="ex", bufs=2)
                        zz = pG.tile([BTW, 1], F32, tag="zz", bufs=2)
                        nc.scalar.activation(ex, g32[bt][:, 0:K], AF.Exp,
                                             bias=nbg, scale=rinv[bt],
                                             accum_out=zz)
                        lnz = pG.tile([BTW, 1], F32, tag="lnz", bufs=2)
                        nc.scalar.activation(lnz, zz, AF.Ln)
                        nc.vector.tensor_sub(bias2[bt], nbg, nb_l[bt])
                        nc.vector.tensor_sub(bias2[bt], bias2[bt], lnz)
                        nc.scalar.activation(rowfix[bt], bias2[bt], AF.Exp)

                # rest of the value stream (reuse-gated; see emit_vtb note)
                for g in range(VTB_BUFS, MT // KTPB):
                    emit_vtb(pV, vtbs, g)

                # ------------- Phase W: dense weights + matmul2 (fp16) ----
                with (
                    tc.tile_pool(name="poolW", bufs=1) as pW,
                    tc.tile_pool(name="psumW", bufs=1, space="PSUM") as psW,
                ):
                    for bt in range(BT):
                        nc.vector.scalar_tensor_tensor(
                            out=wexp[bt], in0=sim[bt],
                            scalar=g32[bt][:, K - 1:K], in1=wexp[bt],
                            op0=ALU.is_ge, op1=ALU.mult)
                    pm = [psW.tile([128, B], F32, tag=f"pm{dt}",
                                   name=f"pm{dt}") for dt in range(CT)]
                    vt16 = None
                    for mt in range(MT):
                        g, kt = mt // KTPB, mt % KTPB
                        if kt == 0:
                            vt16 = pW.tile([128, KTPB, C], F16, tag="vt16",
                                           bufs=2)
                            nc.vector.tensor_copy(vt16, vtbs[g])
                        pwt = psW.tile([128, B], F32, tag="pwt", bufs=3)
                        for bt in range(BT):
                            nc.tensor.matmul(
                                pwt[:, ts(bt, BTW)],
                                lhsT=wexp[bt][:, ts(mt, 128)],
                                rhs=identity, is_transpose=True,
                                start=True, stop=True, skip_group_check=True)
                        wT16 = pW.tile([128, B], F16, tag="wT16", bufs=3)
                        if mt % 2 == 0:
                            nc.vector.tensor_copy(wT16, pwt)
                        else:
                            nc.scalar.copy(wT16, pwt)
                        for dt in range(CT):
                            nc.tensor.matmul(
                                pm[dt], lhsT=vt16[:, kt, ts(dt, 128)],
                                rhs=wT16,
                                start=(mt == 0), stop=(mt == MT - 1),
                                skip_group_check=True)
                    for dt in range(CT):
                        nc.any.tensor_copy(mT[:, dt], pm[dt])

            # ---------------- Phase O: reduce-scatter + broadcast out -----
            with (
                tc.tile_pool(name="poolO", bufs=1) as pO,
                tc.tile_pool(name="psumO", bufs=1, space="PSUM") as psO,
            ):
                for bt in range(BT):
                    pmb = psO.tile([BTW, C], F32, tag="pmb", bufs=2)
                    for dt in range(CT):
                        nc.tensor.matmul(
                            pmb[:, ts(dt, 128)],
                            lhsT=mT[:, dt, ts(bt, BTW)],
                            rhs=identity, is_transpose=True,
                            start=True, stop=True, skip_group_check=True)
                    mrow = pO.tile([BTW, C], BF16, tag="mrow", bufs=2)
                    # fold the local->global softmax correction in here
                    nc.scalar.mul(mrow, pmb, rowfix[bt])
                    nc.sync.dma_start(out=mb_dram[ts(bt, BTW), :], in_=mrow)
                nc.gpsimd.collective_compute(
                    "ReduceScatter", ALU.add, replica_groups=RG,
                    ins=[mb_dram.opt()], outs=[rs_out.opt()])
                mmy = pO.tile([BS, C], BF16, tag="mmy", bufs=1)
                nc.sync.dma_start(out=mmy, in_=rs_out)
                for dt in range(CT):
                    pmt = psO.tile([128, BS], BF16, tag="pmt", bufs=2)
                    nc.tensor.matmul(
                        pmt, lhsT=mmy[:, ts(dt, 128)],
                        rhs=identity16[0:BS, 0:BS], is_transpose=True,
                        start=True, stop=True, skip_group_check=True)
                    nc.any.tensor_copy(mTmy[dt], pmt)
                for oi in range(BS // OPD):
                    ot = pO.tile([128, OPD, CT, HW], F32, tag="ot", bufs=2)
                    for bs_ in range(OPD):
                        b = oi * OPD + bs_
                        for dt in range(CT):
                            col = mTmy[dt][:, b:b + 1]
                            if dt < CT // 2:
                                nc.vector.tensor_scalar_mul(
                                    ot[:, bs_, dt], ones_hw, col)
                            else:
                                nc.scalar.mul(ot[:, bs_, dt], ones_hw, col)
                    nc.sync.dma_start(
                        out=out[oi * OPD:(oi + 1) * OPD].rearrange(
                            "b (ct p) hw -> p b ct hw", p=128),
                        in_=ot)

    nc.compile()
    return nc


_CACHE = {}
TRACE = False
LAST_RESULT = None


def _get(shape_key):
    if shape_key not in _CACHE:
        _CACHE[shape_key] = build(*shape_key)
    return _CACHE[shape_key]


def kernel(x, keys, values, topk, **_ignored):
    K = int(np.asarray(topk))
    B, C, H, W = x.shape
    M, D = keys.shape
    HW = H * W
    nc = _get((B, C, HW, M, K, N_CORES))
    BS, MS = B // N_CORES, M // N_CORES
    x3 = np.ascontiguousarray(x.reshape(B, C, HW)).astype(np.float32, copy=False)
    keys = np.ascontiguousarray(keys).astype(np.float32, copy=False)
    values = np.ascontiguousarray(values).astype(np.float32, copy=False)
    in_maps = [{
        "xs": x3[c * BS:(c + 1) * BS],
        "keys": keys[c * MS:(c + 1) * MS],
        "vals": values[c * MS:(c + 1) * MS],
    } for c in range(N_CORES)]
    global LAST_RESULT
    res = run_bass_kernel_spmd(nc, in_maps, core_ids=list(range(N_CORES)),
                               trace=TRACE)
    LAST_RESULT = res
    outs = [res.results[c]["out"] for c in range(N_CORES)]
    return np.concatenate(outs, axis=0).reshape(B, C, H, W)
